# revision 10
# baseline (speedup 1.0000x reference)
"""MatryoshkaTranscoder TRN2 kernel.

Data-parallel over batch: 8 cores x 512 rows each. Per core:
  phase H: transpose + bf16-split h slice -> h1T/h2T [d, b]
  phase 1: stream W_enc in 512-latent chunks: PE-transpose fp32 -> split
           bf16 hi/lo -> 3-term split matmul (h1*w1 + h2*w1 + h1*w2)
           accumulated in PSUM (+ K=1 bias matmul) -> jump_relu -> z to DRAM
  phase 2: per 128-row tile, per level: exact top-k via max8/match_replace
           (k<=64 direct; k=128 via pair-max tree + threshold + dynamic
           cleanup) -> z_sparse out + z_sparse.T bf16 to DRAM
  phase 3: decode: cast-DMA W_dec4 to bf16, PE-transpose chunks, matmul
           z_sparse.T (stationary) x W_dec4.T (moving) -> recon out
"""
import numpy as np

import concourse.bass as bass
import concourse.mybir as mybir
from concourse import bacc
from concourse.tile import TileContext
from concourse.masks import make_identity

F32 = mybir.dt.float32
BF16 = mybir.dt.bfloat16
AOP = mybir.AluOpType

B = 4096
D = 2048
L = 32768
O = 2048
NCORES = 8
BLOC = B // NCORES          # 512 rows per core
NBT = BLOC // 128           # 4 b-tiles
LCH = 512                   # encoder latent chunk
NLCH = L // LCH             # 64
NDB = D // 128              # 16 d-blocks
LEV_START = [0, 2048, 4096, 8192, 16384]
LEV_END = [2048, 4096, 8192, 16384, 32768]
LEV_K = [8, 16, 32, 64, 128]
CLEAN_ROUNDS = 3            # removal rounds for the L4 tree (max 24 extras)
CBIG = float(2 ** 20)       # count-scaling constant for L4 candidate count
OCH = 512                   # decode output chunk
NOCH = O // OCH             # 4
DLC = 128                   # decode latent chunk (stationary K)
NDLC = L // DLC             # 256
WDSUP = 2048                # wdec cast-DMA superchunk (latent cols)


def build(debug_z=False, debug_l4=False):
    nc = bacc.Bacc("TRN2", target_bir_lowering=False, debug=False)

    h2s = nc.dram_tensor("h2s", [BLOC, D], F32, kind="ExternalInput")
    wenc = nc.dram_tensor("wenc", [L, D], F32, kind="ExternalInput")
    benc = nc.dram_tensor("benc", [L], F32, kind="ExternalInput")
    wdec = nc.dram_tensor("wdec", [O, L], F32, kind="ExternalInput")
    bdec = nc.dram_tensor("bdec", [O], F32, kind="ExternalInput")
    zs_out = nc.dram_tensor("zs", [BLOC, L], F32, kind="ExternalOutput")
    rec_out = nc.dram_tensor("recon", [BLOC, O], F32, kind="ExternalOutput")

    zbuf = nc.dram_tensor("zbuf", [BLOC, L], F32,
                          kind="ExternalOutput" if debug_z else "Internal")
    if debug_l4:
        dbg_thr = nc.dram_tensor("dbg_thr", [128, 8], F32, kind="ExternalOutput")
        dbg_cnt = nc.dram_tensor("dbg_cnt", [128, 1], F32, kind="ExternalOutput")
        dbg_w1 = nc.dram_tensor("dbg_w1", [128, 16384], F32, kind="ExternalOutput")
        dbg_w2 = nc.dram_tensor("dbg_w2", [128, 16384], F32, kind="ExternalOutput")
    zsT = nc.dram_tensor("zsTbuf", [L, BLOC], BF16)

    with TileContext(nc) as tc:
        with tc.tile_pool(name="consts", bufs=1) as cpool:
            identf = cpool.tile([128, 128], F32)
            make_identity(nc, identf[:])
            identb = cpool.tile([128, 128], BF16)
            make_identity(nc, identb[:])
            ones1 = cpool.tile([1, 128], F32)
            nc.vector.memset(ones1[:], 1.0)

            # ---------------- phase H: h transpose + split ----------------
            with tc.tile_pool(name="hmain", bufs=1) as hpool:
                h1T = [hpool.tile([128, BLOC], BF16, tag=f"h1T_{j}",
                                  name=f"h1T_{j}")
                       for j in range(NDB)]
                h2T = [hpool.tile([128, BLOC], BF16, tag=f"h2T_{j}",
                                  name=f"h2T_{j}")
                       for j in range(NDB)]
                with tc.tile_pool(name="hprep", bufs=2) as hp, \
                     tc.tile_pool(name="hps", bufs=2, space="PSUM") as hps:
                    hnat = []
                    for bt in range(NBT):
                        t = hp.tile([128, D], F32, tag=f"hnat{bt}")
                        nc.sync.dma_start(t[:], h2s[bt * 128:(bt + 1) * 128, :])
                        hnat.append(t)
                    for j in range(NDB):
                        ps = hps.tile([128, BLOC], F32, tag="htp")
                        for bt in range(NBT):
                            nc.tensor.transpose(
                                ps[:, bt * 128:(bt + 1) * 128],
                                hnat[bt][:, j * 128:(j + 1) * 128], identf[:])
                        hT = hp.tile([128, BLOC], F32, tag="hTf32")
                        nc.scalar.copy(hT[:], ps[:])
                        nc.vector.tensor_copy(h1T[j][:], hT[:])
                        nc.vector.scalar_tensor_tensor(
                            out=h2T[j][:], in0=hT[:], scalar=1.0, in1=h1T[j][:],
                            op0=AOP.mult, op1=AOP.subtract)

                # ---------------- phase 1: encoder ----------------
                with tc.tile_pool(name="enc", bufs=1) as epool, \
                     tc.tile_pool(name="encps", bufs=2, space="PSUM") as eps, \
                     tc.tile_pool(name="encpsz", bufs=2, space="PSUM") as epz:
                    for lc in range(NLCH):
                        wnat = []
                        for lt in range(4):
                            t = epool.tile([128, D], F32, tag=f"wnat{lt}",
                                           bufs=2)
                            nc.sync.dma_start(
                                t[:],
                                wenc[lc * LCH + lt * 128:
                                     lc * LCH + (lt + 1) * 128, :])
                            wnat.append(t)
                        w1 = []
                        w2 = []
                        for j in range(NDB):
                            ps = eps.tile([128, LCH], F32, tag="wtp")
                            for lt in range(4):
                                nc.tensor.transpose(
                                    ps[:, lt * 128:(lt + 1) * 128],
                                    wnat[lt][:, j * 128:(j + 1) * 128],
                                    identf[:])
                            wtf = epool.tile([128, LCH], F32, tag="wtf",
                                             bufs=4)
                            nc.scalar.copy(wtf[:], ps[:])
                            a = epool.tile([128, LCH], BF16, tag=f"w1_{j}",
                                           bufs=2)
                            b = epool.tile([128, LCH], BF16, tag=f"w2_{j}",
                                           bufs=2)
                            nc.vector.tensor_copy(a[:], wtf[:])
                            nc.gpsimd.tensor_tensor(out=b[:], in0=wtf[:],
                                                    in1=a[:], op=AOP.subtract)
                            w1.append(a)
                            w2.append(b)
                        bstage = epool.tile([1, LCH], F32, tag="bstage",
                                            bufs=2)
                        nc.sync.dma_start(
                            bstage[:], benc[lc * LCH:(lc + 1) * LCH])
                        for bt in range(NBT):
                            psz = epz.tile([128, LCH], F32, tag="psz")
                            bsl = slice(bt * 128, (bt + 1) * 128)
                            for j in range(NDB):
                                nc.tensor.matmul(
                                    psz[:], h1T[j][:, bsl], w1[j][:],
                                    start=(j == 0), stop=False)
                                nc.tensor.matmul(
                                    psz[:], h1T[j][:, bsl], w2[j][:],
                                    start=False, stop=False)
                                nc.tensor.matmul(
                                    psz[:], h2T[j][:, bsl], w1[j][:],
                                    start=False, stop=False)
                            nc.tensor.matmul(
                                psz[:], ones1[:], bstage[:],
                                start=False, stop=True)
                            r = epool.tile([128, LCH], F32, tag="relu",
                                           bufs=2)
                            nc.scalar.activation(
                                r[:], psz[:], mybir.ActivationFunctionType.Relu)
                            zt = epool.tile([128, LCH], F32, tag="zt", bufs=2)
                            nc.vector.scalar_tensor_tensor(
                                out=zt[:], in0=psz[:], scalar=1.0, in1=r[:],
                                op0=AOP.is_gt, op1=AOP.add)
                            nc.sync.dma_start(
                                zbuf[bsl, lc * LCH:(lc + 1) * LCH], zt[:])

            # ---------------- phase 2: topk ----------------
            with tc.tile_pool(name="tk", bufs=1) as tpool, \
                 tc.tile_pool(name="tkps", bufs=2, space="PSUM") as tps:
                for bt in range(NBT):
                    bsl = slice(bt * 128, (bt + 1) * 128)
                    for lv in range(5):
                        s0, s1, k = LEV_START[lv], LEV_END[lv], LEV_K[lv]
                        W = s1 - s0
                        zt = tpool.tile([128, 16384], F32, tag="zt_lv",
                                        name="ztlv")[:, :W]
                        nc.sync.dma_start(zt[:], zbuf[bsl, s0:s1])
                        work = tpool.tile([128, 16384], F32, tag="work_lv",
                                          name="worklv")[:, :W]
                        mx = tpool.tile([128, 8], F32, tag="mx")
                        if k <= 64:
                            if k == 8:
                                nc.vector.max(out=mx[:], in_=zt[:])
                            else:
                                nc.vector.tensor_copy(work[:], zt[:])
                                for r_ in range(k // 8):
                                    nc.vector.max(out=mx[:], in_=work[:])
                                    if r_ < k // 8 - 1:
                                        nc.vector.match_replace(
                                            out=work[:], in_to_replace=mx[:],
                                            in_values=work[:], imm_value=0.0)
                            thr = mx[:, 7:8]
                            sel_in, sel_op = zt, AOP.is_ge
                            sel_scalar = thr
                        else:
                            # pair-max tree (built inside `work`) to m3
                            # (2048), topk-128 of m3 for a lower-bound
                            # threshold
                            m1 = work[:, 0:W // 2]
                            nc.vector.tensor_tensor(
                                out=m1, in0=zt[:, 0:W:2], in1=zt[:, 1:W:2],
                                op=AOP.max)
                            m2 = work[:, W // 2:W // 2 + W // 4]
                            nc.vector.tensor_tensor(
                                out=m2, in0=m1[:, 0:W // 2:2],
                                in1=m1[:, 1:W // 2:2], op=AOP.max)
                            m3 = work[:, W // 2 + W // 4:
                                      W // 2 + W // 4 + W // 8]
                            nc.vector.tensor_tensor(
                                out=m3, in0=m2[:, 0:W // 4:2],
                                in1=m2[:, 1:W // 4:2], op=AOP.max)
                            for r_ in range(16):
                                nc.vector.max(out=mx[:], in_=m3)
                                if r_ < 15:
                                    nc.vector.match_replace(
                                        out=m3, in_to_replace=mx[:],
                                        in_values=m3, imm_value=0.0)
                            thr = mx[:, 7:8]
                            # q = 8 - zt (small shift keeps full fp32
                            # resolution for the ordering array)
                            q = tpool.tile([128, 16384], F32, tag="q_lv",
                                           name="qlv")[:, :W]
                            nc.vector.tensor_scalar(
                                q[:], zt[:], -1.0, 8.0,
                                op0=AOP.mult, op1=AOP.add)
                            # work = candidate mask; cnt = exact count
                            cnt = tpool.tile([128, 1], F32, tag="cnt")
                            nc.vector.tensor_scalar(
                                work[:], zt[:], thr, None, op0=AOP.is_ge)
                            nc.vector.tensor_reduce(
                                out=cnt[:], in_=work[:],
                                axis=mybir.AxisListType.X, op=AOP.add)
                            # work = mask * (8 - z): largest = smallest cand
                            nc.vector.tensor_tensor(
                                out=work[:], in0=work[:], in1=q[:],
                                op=AOP.mult)
                            if debug_l4 and bt == 0:
                                nc.sync.dma_start(dbg_thr[:, :], mx[:])
                                nc.sync.dma_start(dbg_cnt[:, :], cnt[:])
                                dw1 = tpool.tile([128, 16384], F32,
                                                 tag="q_lv", name="dw1")
                                nc.vector.tensor_copy(dw1[:], work[:])
                                nc.sync.dma_start(dbg_w1[:, :], dw1[:])
                            # krem[j] = c - 128.5 - j ; slot j of round r
                            # removes the (8r+j+1)-th smallest candidate iff
                            # 8r + j < c - 128, i.e. krem > 0
                            krem = tpool.tile([128, 8], F32, tag="krem")
                            for j in range(8):
                                nc.vector.memset(
                                    krem[:, j:j + 1], -(128.5 + j))
                            nc.vector.tensor_add(
                                krem[:], krem[:],
                                cnt.to_broadcast([128, 8]))
                            mxs = tpool.tile([128, 16], F32, tag="mxs")
                            for r_ in range(CLEAN_ROUNDS):
                                nc.vector.memset(mxs[:], 0.0)
                                nc.vector.max(out=mxs[:, 0:8], in_=work[:])
                                done = tpool.tile([128, 8], mybir.dt.uint32,
                                                  tag="done")
                                nc.vector.tensor_scalar(
                                    done[:], krem[:], 0.0, scalar2=None,
                                    op0=AOP.is_le)
                                nc.vector.copy_predicated(
                                    mxs[:, 0:8], done[:], mxs[:, 8:16])
                                nc.vector.tensor_scalar(
                                    krem[:], krem[:], 8.0, scalar2=None,
                                    op0=AOP.subtract)
                                nc.vector.match_replace(
                                    out=work[:], in_to_replace=mxs[:, 0:8],
                                    in_values=work[:], imm_value=0.0)
                            if debug_l4 and bt == 0:
                                nc.sync.dma_start(dbg_w2[:, :], work[:])
                            sel_in, sel_op = work, AOP.is_gt
                            sel_scalar = 0.0
                        # selection mask -> zs chunks; cast + transpose for
                        # decode
                        for c0 in range(0, W, 512):
                            zsc = tpool.tile([128, 512], F32, tag="zsc",
                                             bufs=2)
                            nc.vector.scalar_tensor_tensor(
                                out=zsc[:], in0=sel_in[:, c0:c0 + 512],
                                scalar=sel_scalar, in1=zt[:, c0:c0 + 512],
                                op0=sel_op, op1=AOP.mult)
                            nc.sync.dma_start(
                                zs_out[bsl, s0 + c0:s0 + c0 + 512], zsc[:])
                            zsb = tpool.tile([128, 512], BF16, tag="zsb",
                                             bufs=2)
                            nc.vector.tensor_copy(zsb[:], zsc[:])
                            for q in range(4):
                                pst = tps.tile([128, 128], BF16, tag="zstp")
                                nc.tensor.transpose(
                                    pst[:], zsb[:, q * 128:(q + 1) * 128],
                                    identb[:])
                                ztp = tpool.tile([128, 128], BF16, tag="ztp",
                                                 bufs=2)
                                nc.scalar.copy(ztp[:], pst[:])
                                lrow = s0 + c0 + q * 128
                                nc.sync.dma_start(
                                    zsT[lrow:lrow + 128, bsl], ztp[:])

            # ---------------- phase 3: decode ----------------
            with tc.tile_pool(name="dec", bufs=1) as dpool, \
                 tc.tile_pool(name="decps", bufs=2, space="PSUM") as dps, \
                 tc.tile_pool(name="decrec", bufs=1, space="PSUM") as drec:
                for oc in range(NOCH):
                    osl = slice(oc * OCH, (oc + 1) * OCH)
                    psr = [drec.tile([128, OCH], F32, tag=f"psr{bt}",
                                     name=f"psr{bt}")
                           for bt in range(NBT)]
                    for sc in range(L // WDSUP):
                        wdn = []
                        for ot in range(OCH // 128):
                            t = dpool.tile([128, WDSUP], BF16,
                                           tag=f"wdn{ot}", bufs=2)
                            nc.gpsimd.dma_start(
                                t[:],
                                wdec[oc * OCH + ot * 128:
                                     oc * OCH + (ot + 1) * 128,
                                     sc * WDSUP:(sc + 1) * WDSUP])
                            wdn.append(t)
                        for li in range(WDSUP // 128):
                            lc = sc * (WDSUP // 128) + li
                            psw = dps.tile([128, OCH], BF16, tag="pswd")
                            for ot in range(OCH // 128):
                                nc.tensor.transpose(
                                    psw[:, ot * 128:(ot + 1) * 128],
                                    wdn[ot][:, li * 128:(li + 1) * 128],
                                    identb[:])
                            wdT = dpool.tile([128, OCH], BF16, tag="wdT",
                                             bufs=3)
                            nc.scalar.copy(wdT[:], psw[:])
                            zt4 = dpool.tile([128, BLOC], BF16, tag="zt4",
                                             bufs=3)
                            nc.sync.dma_start(
                                zt4[:], zsT[lc * 128:(lc + 1) * 128, :])
                            for bt in range(NBT):
                                nc.tensor.matmul(
                                    psr[bt][:],
                                    zt4[:, bt * 128:(bt + 1) * 128],
                                    wdT[:], start=(lc == 0), stop=False)
                    bstage = dpool.tile([1, OCH], F32, tag="bdstage", bufs=2)
                    nc.sync.dma_start(bstage[:], bdec[osl])
                    for bt in range(NBT):
                        nc.tensor.matmul(psr[bt][:], ones1[:], bstage[:],
                                         start=False, stop=True)
                        rec = dpool.tile([128, OCH], F32, tag="rec", bufs=2)
                        nc.scalar.copy(rec[:], psr[bt][:])
                        nc.sync.dma_start(
                            rec_out[bt * 128:(bt + 1) * 128, osl], rec[:])

    nc.compile()
    return nc


_NC_CACHE = None
LAST_RESULT = None


def _get_nc():
    global _NC_CACHE
    if _NC_CACHE is None:
        _NC_CACHE = build()
    return _NC_CACHE


def kernel(**inputs):
    from concourse.bass_utils import run_bass_kernel_spmd

    h_2 = np.ascontiguousarray(np.asarray(inputs["h_2"], dtype=np.float32))
    wenc = np.ascontiguousarray(np.asarray(inputs["W_enc"], dtype=np.float32))
    benc = np.ascontiguousarray(np.asarray(inputs["b_enc"], dtype=np.float32))
    wdec = np.ascontiguousarray(np.asarray(inputs["W_dec4"], dtype=np.float32))
    bdec = np.ascontiguousarray(np.asarray(inputs["b_dec4"], dtype=np.float32))

    nc = _get_nc()
    in_maps = []
    for i in range(NCORES):
        in_maps.append({
            "h2s": h_2[i * BLOC:(i + 1) * BLOC],
            "wenc": wenc,
            "benc": benc,
            "wdec": wdec,
            "bdec": bdec,
        })
    global LAST_RESULT
    r = run_bass_kernel_spmd(nc, in_maps, core_ids=list(range(NCORES)))
    LAST_RESULT = r
    recon = np.concatenate([r.results[i]["recon"] for i in range(NCORES)], 0)
    zs = np.concatenate([r.results[i]["zs"] for i in range(NCORES)], 0)
    return recon, zs
